# revision 7
# baseline (speedup 1.0000x reference)
"""Windowed multi-head attention (Swin-style) Bass kernel for Trainium2.

Full inputs -> shard over 8 NeuronCores (data-parallel over windows) -> full output.

Math per window w (n=60 tokens, d=256, h=8 heads, dh=32):
  qkv = x_w @ w_qkv ; sim = scale*q_h @ k_h^T + bias_h ; attn = softmax(sim)
  out_w = concat_h(attn @ v_h) @ w_out

Device formulation v2 (per group of 2 windows, 120 token rows; groups in
quads for fat qkT streams; PE-transpose instead of DMA-transpose; bias
applied as et = exp(sim) * exp(bias) so every sim matmul is start=True —
no PSUM has_written preload trick, which raced on the first group):
  - host pre-transposes x -> xT [128, 2, 4, 120] per quad (bf16), pre-scales
    w_q by dh^-0.5, precomputes expb = exp(bias^T) with ZEROS in the
    cross-window blocks (masks the 2-window batching for free)
  - qT, kT head-dim-major via matmul(lhsT=w_slice, rhs=xT), N=480 per quad
  - q evacuated DENSE to SBUF (1 copy), then scattered into the
    block-diagonal qbd staging tiles by small SBUF->SBUF DMAs (DMA engines
    are idle; saves ~1us/group of Vector/Scalar block-copy time)
  - simT for 4 heads in ONE matmul: lhsT = kT chunk [128, 128-col padded
    for fast weight load], rhs = qT block-diag [128, 4*120]
  - et = exp(simT) [scalar] ; et *= expb [gpsimd, SBUF-only engine]
  - av per head: matmul(lhsT=et_h [120,128 padded], rhs=[v_h | ones])
    -> [128, 33]: cols 0-31 unnormalized attn@v token-major, col 32 = denom
  - normalize: ao = av * (1/s) (one broadcast multiply, DVE)
  - PE-mode transpose ao -> aoT (bf16 PSUM), evac to SBUF, then
    proj: matmul(lhsT=aoT, rhs=w_out) -> [120, 256] -> DMA out
"""

import os
from contextlib import ExitStack

import numpy as np
import ml_dtypes

os.environ.setdefault("JAX_COMPILATION_CACHE_DIR", "/tmp/jaxcache")
os.environ.setdefault("JAX_PERSISTENT_CACHE_MIN_COMPILE_TIME_SECS", "2")

N_CORES = 8
WH, WW = 6, 10
N_TOK = WH * WW          # 60 tokens per window
DIM = 256
HEADS = 8
DH = 32
B_WINDOWS = 16 * 16 * 16  # 4096
GROUPS_TOTAL = B_WINDOWS // 2        # 2048 groups of 2 windows
NGROUPS = GROUPS_TOTAL // N_CORES    # 256 per core
GT = 2 * N_TOK           # 120 rows per group

BF16 = ml_dtypes.bfloat16

LAST_RESULT = None  # BassKernelResults from the most recent run (for test.py)

# Bumped on every kernel revision: becomes the shape of a dummy input so the
# PJRT-level executable cache (keyed on jit signature, not the embedded BIR)
# can never serve a stale NEFF for a changed kernel.
KERNEL_REV = 6

_CACHE = {}


def _build_bass(ngroups: int):
    import concourse.bass as bass
    import concourse.tile as tile
    from concourse import bacc, mybir
    from concourse.masks import make_identity

    fp32 = mybir.dt.float32
    bf16 = mybir.dt.bfloat16
    Copy = mybir.ActivationFunctionType.Copy
    Exp = mybir.ActivationFunctionType.Exp

    nc = bacc.Bacc("TRN2", debug=False, enable_asserts=False)

    nquads = ngroups // 4
    xt_d = nc.dram_tensor("xt", [nquads, 128, 2, 4, GT], bf16, kind="ExternalInput").ap()
    wqkv_d = nc.dram_tensor("wqkv", [2, 128, 768], bf16, kind="ExternalInput").ap()
    wout_d = nc.dram_tensor("wout", [2, 128, 256], bf16, kind="ExternalInput").ap()
    expb_d = nc.dram_tensor("expb", [GT, HEADS, GT], bf16, kind="ExternalInput").ap()
    out_d = nc.dram_tensor("out", [ngroups, GT, 256], fp32, kind="ExternalOutput").ap()
    cb_d = nc.dram_tensor("cachebust", [1, KERNEL_REV], bf16, kind="ExternalInput").ap()

    with tile.TileContext(nc) as tc, ExitStack() as ctx:
        consts = ctx.enter_context(tc.tile_pool(name="consts", bufs=1))

        wqkv_sb = consts.tile([128, 2, 768], bf16)
        for kh in range(2):
            nc.gpsimd.dma_start(out=wqkv_sb[:, kh, :], in_=wqkv_d[kh])
        wout_sb = consts.tile([128, 2, 256], bf16)
        for kh in range(2):
            nc.gpsimd.dma_start(out=wout_sb[:, kh, :], in_=wout_d[kh])
        expb_sb = consts.tile([GT, HEADS, GT], bf16)
        nc.gpsimd.dma_start(out=expb_sb, in_=expb_d)
        cb_sb = consts.tile([1, KERNEL_REV], bf16)
        nc.gpsimd.dma_start(out=cb_sb, in_=cb_d)

        ident = consts.tile([128, 128], bf16)
        make_identity(nc, ident)

        # block-diag qT staging per quad parity: [p, c(chunk), m(diag), gq, t];
        # zeroed once, only diagonal blocks rewritten each quad (by SBUF->SBUF
        # DMAs issued from the idle vector/gpsimd sequencers)
        qbd = [consts.tile([128, 2, 4, 4, GT], bf16, name=f"qbd{par}")
               for par in range(2)]
        for par in range(2):
            nc.vector.memset(qbd[par], 0.0)

        xpool = ctx.enter_context(tc.tile_pool(name="xp", bufs=3))
        qpool = ctx.enter_context(tc.tile_pool(name="qp", bufs=2))
        ktpool = ctx.enter_context(tc.tile_pool(name="kt", bufs=2))
        etpool = ctx.enter_context(tc.tile_pool(name="et", bufs=4))
        vpool = ctx.enter_context(tc.tile_pool(name="vp", bufs=3))
        aopool = ctx.enter_context(tc.tile_pool(name="ao", bufs=3))
        aotpool = ctx.enter_context(tc.tile_pool(name="aot", bufs=3))
        rpool = ctx.enter_context(tc.tile_pool(name="rp", bufs=4))
        opool = ctx.enter_context(tc.tile_pool(name="op", bufs=4))

        pqk = ctx.enter_context(tc.tile_pool(name="pqk", bufs=2, space="PSUM"))
        pst = ctx.enter_context(tc.tile_pool(name="pst", bufs=2, space="PSUM"))
        pavt = ctx.enter_context(tc.tile_pool(name="pavt", bufs=2, space="PSUM"))
        pmisc = ctx.enter_context(tc.tile_pool(name="pmisc", bufs=2, space="PSUM"))

        for q in range(nquads):
            par = q % 2
            xt = xpool.tile([128, 2, 4, GT], bf16)  # [p, kh, gq, t]
            nc.sync.dma_start(out=xt, in_=xt_d[q])

            # fat qkT matmuls over the quad: chunks 0,1 = q ; 2,3 = k
            kt = ktpool.tile([128, 2, 4, 128], bf16)  # col-padded for FWL
            nc.gpsimd.memset(kt[:, :, :, GT:128], 0.0)
            for c in range(4):
                ps = pqk.tile([128, 4, GT], fp32, tag="ps")
                for kh in range(2):
                    nc.tensor.matmul(
                        ps.rearrange("p a b -> p (a b)"),
                        lhsT=wqkv_sb[:, kh, c * 128:(c + 1) * 128],
                        rhs=xt[:, kh].rearrange("p a b -> p (a b)"),
                        start=(kh == 0),
                        stop=(kh == 1),
                    )
                if c < 2:
                    # q chunk: dense evac into the shared qd tile
                    if c == 0:
                        qd = qpool.tile([128, 2, 4, GT], bf16, tag="qd")
                        nc.vector.tensor_copy(out=qd[:, 0], in_=ps)
                    else:
                        nc.scalar.activation(out=qd[:, 1], in_=ps, func=Copy)
                        # block-diag scatter: 4 SBUF->SBUF DMAs (both chunks)
                        for m in range(4):
                            eng = nc.sync if m < 2 else nc.gpsimd
                            eng.dma_start(
                                out=qbd[par][m * DH:(m + 1) * DH, :, m, :, :],
                                in_=qd[m * DH:(m + 1) * DH, :, :, :],
                            )
                else:
                    if c == 2:
                        nc.vector.tensor_copy(out=kt[:, 0, :, 0:GT], in_=ps)
                    else:
                        nc.scalar.activation(out=kt[:, 1, :, 0:GT], in_=ps,
                                             func=Copy)

            for gq in range(4):
                g = 4 * q + gq
                # v token-major with interleaved ones col: [120, h, 33]
                pv = pmisc.tile([GT, 256], fp32, tag="m")
                for kh in range(2):
                    nc.tensor.matmul(
                        pv,
                        lhsT=xt[:, kh, gq, :],
                        rhs=wqkv_sb[:, kh, 512:768],
                        start=(kh == 0),
                        stop=(kh == 1),
                    )
                v1 = vpool.tile([GT, HEADS, DH + 1], bf16)
                nc.gpsimd.memset(v1[:, :, DH:DH + 1], 1.0)
                nc.vector.tensor_copy(
                    out=v1[:, :, 0:DH],
                    in_=pv.rearrange("p (h d) -> p h d", h=HEADS),
                )

                # fat simT per chunk: [120 (j), 4*120 (h,i)], then
                # et = exp(sim) * exp(bias)  (expb has zeros cross-window)
                ets = []
                for c in range(2):
                    ps_sim = pst.tile([128, 4, GT], fp32, tag="st")
                    nc.tensor.matmul(
                        ps_sim.rearrange("p a b -> p (a b)"),
                        lhsT=kt[:, c, gq, :],
                        rhs=qbd[par][:, c, :, gq, :],
                        start=True, stop=True,
                    )
                    et = etpool.tile([GT, 4, 128], bf16, tag="et")
                    nc.gpsimd.memset(et[:, :, GT:128], 0.0)
                    nc.scalar.activation(out=et[:, :, 0:GT], in_=ps_sim[0:GT],
                                         func=Exp)
                    eng = nc.vector if c == 0 else nc.gpsimd
                    eng.tensor_mul(
                        et[:, :, 0:GT], et[:, :, 0:GT],
                        expb_sb[:, 4 * c:4 * c + 4, :],
                    )
                    ets.append(et)

                # av: 8 small matmuls; lhsT 128-col padded for FWL
                pav = pavt.tile([128, HEADS, DH + 1], fp32, tag="avt")
                for h in range(8):
                    nc.tensor.matmul(
                        pav[:, h, :],
                        lhsT=ets[h // 4][:, h % 4, :],
                        rhs=v1[:, h, :],
                        start=True, stop=True,
                    )

                # normalize: r = 1/s ; ao = av * r (single broadcast multiply)
                r = rpool.tile([GT, HEADS, 1], fp32)
                nc.vector.reciprocal(out=r, in_=pav[0:GT, :, DH:DH + 1])
                ao = aopool.tile([128, 2, 128], bf16)
                nc.gpsimd.memset(ao[96:128, :, :], 0.0)
                aov = ao[0:GT].rearrange("p a (b d) -> p (a b) d", d=DH)
                rb = bass.AP(tensor=r.tensor, offset=r.offset,
                             ap=[r.ap[0], r.ap[1], [0, DH]])
                nc.vector.tensor_mul(aov, pav[0:GT, :, 0:DH], rb)

                # PE-mode transpose ao -> aoT (bf16 PSUM), evac to SBUF
                ptr = pavt.tile([128, 2, 128], bf16, tag="avt")
                for kh in range(2):
                    nc.tensor.transpose(ptr[:, kh, :], ao[:, kh, :], ident)
                aot = aotpool.tile([128, 2, 128], bf16)
                nc.vector.tensor_copy(out=aot, in_=ptr)

                # out projection
                po = pmisc.tile([128, 256], fp32, tag="m")
                for kh in range(2):
                    nc.tensor.matmul(
                        po,
                        lhsT=aot[:, kh, :],
                        rhs=wout_sb[:, kh, :],
                        start=(kh == 0),
                        stop=(kh == 1),
                    )
                osb = opool.tile([GT, 256], fp32)
                if gq % 2 == 0:
                    nc.scalar.activation(out=osb, in_=po[0:GT], func=Copy)
                else:
                    nc.vector.tensor_copy(out=osb, in_=po[0:GT])
                nc.sync.dma_start(out=out_d[g], in_=osb)

    nc.compile()
    return nc


def _get_bass(ngroups: int):
    if ngroups not in _CACHE:
        _CACHE[ngroups] = _build_bass(ngroups)
    return _CACHE[ngroups]


def _host_prep(x, w_qkv, w_out, bias_table, rel_pos_indices):
    x = np.asarray(x, dtype=np.float32)
    w_qkv = np.asarray(w_qkv, dtype=np.float32)
    w_out = np.asarray(w_out, dtype=np.float32)
    bias_table = np.asarray(bias_table, dtype=np.float32)
    rel_pos_indices = np.asarray(rel_pos_indices)

    nquads_total = GROUPS_TOTAL // 4
    xg = x.reshape(GROUPS_TOTAL, GT, DIM)
    xT = np.ascontiguousarray(xg.transpose(0, 2, 1)).astype(BF16)
    # [quads, 4(gq), 2(kh), 128(p), GT] -> [quads, 128, 2, 4, GT]
    xT = xT.reshape(nquads_total, 4, 2, 128, GT).transpose(0, 3, 2, 1, 4)
    xT = np.ascontiguousarray(xT)

    wq = w_qkv.copy()
    wq[:, :DIM] *= DH ** -0.5
    wqkv_h = np.ascontiguousarray(wq.reshape(2, 128, 768)).astype(BF16)
    wout_h = np.ascontiguousarray(w_out.reshape(2, 128, 256)).astype(BF16)

    bias = bias_table[rel_pos_indices]        # [60, 60, H]  (i, j, h)
    eb = np.exp(bias.transpose(1, 2, 0))      # [j, h, i]
    expb = np.zeros((GT, HEADS, GT), dtype=np.float32)
    for blk in range(2):
        expb[blk * N_TOK:(blk + 1) * N_TOK, :, blk * N_TOK:(blk + 1) * N_TOK] = eb
    expb_h = expb.astype(BF16)
    return xT, wqkv_h, wout_h, expb_h


def kernel(x, w_qkv, w_out, bias_table, rel_pos_indices):
    global LAST_RESULT
    from concourse.bass_utils import run_bass_kernel_spmd

    xT, wqkv_h, wout_h, expb_h = _host_prep(
        x, w_qkv, w_out, bias_table, rel_pos_indices
    )

    nq = NGROUPS // 4
    nc = _get_bass(NGROUPS)
    in_maps = []
    for c in range(N_CORES):
        in_maps.append({
            "xt": np.ascontiguousarray(xT[c * nq:(c + 1) * nq]),
            "wqkv": wqkv_h,
            "wout": wout_h,
            "expb": expb_h,
            "cachebust": np.zeros((1, KERNEL_REV), dtype=BF16),
        })

    res = run_bass_kernel_spmd(
        nc, in_maps, core_ids=list(range(N_CORES)),
        trace=bool(int(os.environ.get("KERNEL_TRACE", "0"))),
    )
    LAST_RESULT = res

    out = np.concatenate([res.results[c]["out"] for c in range(N_CORES)], axis=0)
    out = out.reshape(16, 16, 16, WH, WW, DIM).astype(np.float32)
    return out


# revision 9
# speedup vs baseline: 1.5252x; 1.5252x over previous
"""Windowed multi-head attention (Swin-style) Bass kernel for Trainium2.

Full inputs -> shard over 8 NeuronCores (data-parallel over windows) -> full output.

Math per window w (n=60 tokens, d=256, h=8 heads, dh=32):
  qkv = x_w @ w_qkv ; sim = scale*q_h @ k_h^T + bias_h ; attn = softmax(sim)
  out_w = concat_h(attn @ v_h) @ w_out

Device formulation v2 (per group of 2 windows, 120 token rows; groups in
quads for fat qkT streams; PE-transpose instead of DMA-transpose; bias
applied as et = exp(sim) * exp(bias) so every sim matmul is start=True —
no PSUM has_written preload trick, which raced on the first group):
  - host pre-transposes x -> xT [128, 2, 4, 120] per quad (bf16), pre-scales
    w_q by dh^-0.5, precomputes expb = exp(bias^T) with ZEROS in the
    cross-window blocks (masks the 2-window batching for free)
  - qT, kT head-dim-major via matmul(lhsT=w_slice, rhs=xT), N=480 per quad
  - q evacuated DENSE to SBUF (1 copy), then scattered into the
    block-diagonal qbd staging tiles by small SBUF->SBUF DMAs (DMA engines
    are idle; saves ~1us/group of Vector/Scalar block-copy time)
  - simT for 4 heads in ONE matmul: lhsT = kT chunk [128, 128-col padded
    for fast weight load], rhs = qT block-diag [128, 4*120]
  - et = exp(simT) [scalar] ; et *= expb [gpsimd, SBUF-only engine]
  - av per head: matmul(lhsT=et_h [120,128 padded], rhs=[v_h | ones])
    -> [128, 33]: cols 0-31 unnormalized attn@v token-major, col 32 = denom
  - normalize: ao = av * (1/s) (one broadcast multiply, DVE)
  - PE-mode transpose ao -> aoT (bf16 PSUM), evac to SBUF, then
    proj: matmul(lhsT=aoT, rhs=w_out) -> [120, 256] -> DMA out
"""

import os
from contextlib import ExitStack

import numpy as np
import ml_dtypes

os.environ.setdefault("JAX_COMPILATION_CACHE_DIR", "/tmp/jaxcache")
os.environ.setdefault("JAX_PERSISTENT_CACHE_MIN_COMPILE_TIME_SECS", "2")

N_CORES = 8
WH, WW = 6, 10
N_TOK = WH * WW          # 60 tokens per window
DIM = 256
HEADS = 8
DH = 32
B_WINDOWS = 16 * 16 * 16  # 4096
GROUPS_TOTAL = B_WINDOWS // 2        # 2048 groups of 2 windows
NGROUPS = GROUPS_TOTAL // N_CORES    # 256 per core
GT = 2 * N_TOK           # 120 rows per group

BF16 = ml_dtypes.bfloat16

LAST_RESULT = None  # BassKernelResults from the most recent run (for test.py)

# Bumped on every kernel revision: becomes the shape of a dummy input so the
# PJRT-level executable cache (keyed on jit signature, not the embedded BIR)
# can never serve a stale NEFF for a changed kernel.
KERNEL_REV = 7

_CACHE = {}


def _build_bass(ngroups: int):
    import concourse.bass as bass
    import concourse.tile as tile
    from concourse import bacc, mybir
    from concourse.masks import make_identity

    fp32 = mybir.dt.float32
    bf16 = mybir.dt.bfloat16
    Copy = mybir.ActivationFunctionType.Copy
    Exp = mybir.ActivationFunctionType.Exp

    nc = bacc.Bacc("TRN2", debug=False, enable_asserts=False)

    nquads = ngroups // 4
    xt_d = nc.dram_tensor("xt", [nquads, 128, 2, 4, GT], bf16, kind="ExternalInput").ap()
    wqkv_d = nc.dram_tensor("wqkv", [2, 128, 768], bf16, kind="ExternalInput").ap()
    wout_d = nc.dram_tensor("wout", [2, 128, 256], bf16, kind="ExternalInput").ap()
    expb_d = nc.dram_tensor("expb", [GT, HEADS, GT], bf16, kind="ExternalInput").ap()
    out_d = nc.dram_tensor("out", [ngroups, GT, 256], fp32, kind="ExternalOutput").ap()
    cb_d = nc.dram_tensor("cachebust", [1, KERNEL_REV], bf16, kind="ExternalInput").ap()

    with tile.TileContext(nc) as tc, ExitStack() as ctx:
        consts = ctx.enter_context(tc.tile_pool(name="consts", bufs=1))

        wqkv_sb = consts.tile([128, 2, 768], bf16)
        for kh in range(2):
            nc.gpsimd.dma_start(out=wqkv_sb[:, kh, :], in_=wqkv_d[kh])
        wout_sb = consts.tile([128, 2, 256], bf16)
        for kh in range(2):
            nc.gpsimd.dma_start(out=wout_sb[:, kh, :], in_=wout_d[kh])
        expb_sb = consts.tile([GT, HEADS, GT], bf16)
        nc.gpsimd.dma_start(out=expb_sb, in_=expb_d)
        cb_sb = consts.tile([1, KERNEL_REV], bf16)
        nc.gpsimd.dma_start(out=cb_sb, in_=cb_d)

        ident = consts.tile([128, 128], bf16)
        make_identity(nc, ident)

        # block-diag qT staging per quad parity: [p, c(chunk), m(diag), gq, t];
        # zeroed once, only diagonal blocks rewritten each quad (by SBUF->SBUF
        # DMAs issued from the idle vector/gpsimd sequencers)
        qbd = [consts.tile([128, 2, 4, 4, GT], bf16, name=f"qbd{par}")
               for par in range(2)]
        for par in range(2):
            nc.vector.memset(qbd[par], 0.0)

        xpool = ctx.enter_context(tc.tile_pool(name="xp", bufs=3))
        qpool = ctx.enter_context(tc.tile_pool(name="qp", bufs=2))
        ktpool = ctx.enter_context(tc.tile_pool(name="kt", bufs=2))
        etpool = ctx.enter_context(tc.tile_pool(name="et", bufs=6))
        vpool = ctx.enter_context(tc.tile_pool(name="vp", bufs=3))
        aopool = ctx.enter_context(tc.tile_pool(name="ao", bufs=3))
        aotpool = ctx.enter_context(tc.tile_pool(name="aot", bufs=4))
        rpool = ctx.enter_context(tc.tile_pool(name="rp", bufs=6))
        opool = ctx.enter_context(tc.tile_pool(name="op", bufs=6))

        # PSUM: 8 banks, every group-level pool double-buffered (B=2) by
        # packing each group's outputs into ONE bank-sized tile
        pqk = ctx.enter_context(tc.tile_pool(name="pqk", bufs=2, space="PSUM"))
        pst = ctx.enter_context(tc.tile_pool(name="pst", bufs=2, space="PSUM"))
        pavt = ctx.enter_context(tc.tile_pool(name="pavt", bufs=2, space="PSUM"))
        pmisc = ctx.enter_context(tc.tile_pool(name="pmisc", bufs=2, space="PSUM"))

        # pre-initialize rotating SBUF buffers once so the steady-state loop
        # carries no memsets (stale pad/tail values are finite and provably
        # never read in real output rows/cols)
        for _ in range(6):
            et0 = etpool.tile([GT, 4, 128], bf16, tag="et")
            nc.gpsimd.memset(et0, 0.0)
        for _ in range(2):
            kt0 = ktpool.tile([128, 2, 4, 128], bf16, tag="kt")
            nc.gpsimd.memset(kt0, 0.0)
        for _ in range(3):
            ao0 = aopool.tile([128, 2, 128], bf16, tag="ao")
            nc.gpsimd.memset(ao0, 0.0)
        for _ in range(3):
            v10 = vpool.tile([GT, HEADS, DH + 1], bf16, tag="v1")
            nc.gpsimd.memset(v10, 1.0)

        for q in range(nquads):
            par = q % 2
            xt = xpool.tile([128, 2, 4, GT], bf16)  # [p, kh, gq, t]
            nc.sync.dma_start(out=xt, in_=xt_d[q])

            # fat qkT matmuls over the quad: chunks 0,1 = q ; 2,3 = k
            kt = ktpool.tile([128, 2, 4, 128], bf16, tag="kt")  # col-padded (FWL)
            for c in range(4):
                ps = pqk.tile([128, 4, GT], fp32, tag="ps")
                for kh in range(2):
                    nc.tensor.matmul(
                        ps.rearrange("p a b -> p (a b)"),
                        lhsT=wqkv_sb[:, kh, c * 128:(c + 1) * 128],
                        rhs=xt[:, kh].rearrange("p a b -> p (a b)"),
                        start=(kh == 0),
                        stop=(kh == 1),
                    )
                if c < 2:
                    # q chunk: dense evac into the shared qd tile
                    if c == 0:
                        qd = qpool.tile([128, 2, 4, GT], bf16, tag="qd")
                        nc.vector.tensor_copy(out=qd[:, 0], in_=ps)
                    else:
                        nc.scalar.activation(out=qd[:, 1], in_=ps, func=Copy)
                        # block-diag scatter: 4 SBUF->SBUF DMAs (both chunks)
                        for m in range(4):
                            eng = nc.sync if m < 2 else nc.gpsimd
                            eng.dma_start(
                                out=qbd[par][m * DH:(m + 1) * DH, :, m, :, :],
                                in_=qd[m * DH:(m + 1) * DH, :, :, :],
                            )
                else:
                    if c == 2:
                        nc.vector.tensor_copy(out=kt[:, 0, :, 0:GT], in_=ps)
                    else:
                        nc.scalar.activation(out=kt[:, 1, :, 0:GT], in_=ps,
                                             func=Copy)

            for gq in range(4):
                g = 4 * q + gq
                # v token-major; ones col pre-set at init (never overwritten)
                pm = pmisc.tile([128, 2, 256], fp32, tag="m")
                pv = pm[0:GT, 0]
                for kh in range(2):
                    nc.tensor.matmul(
                        pv,
                        lhsT=xt[:, kh, gq, :],
                        rhs=wqkv_sb[:, kh, 512:768],
                        start=(kh == 0),
                        stop=(kh == 1),
                    )
                v1 = vpool.tile([GT, HEADS, DH + 1], bf16, tag="v1")
                nc.vector.tensor_copy(
                    out=v1[:, :, 0:DH],
                    in_=pv.rearrange("p (h d) -> p h d", h=HEADS),
                )

                # fat simT per chunk: [120 (j), 4*120 (h,i)], then
                # et = exp(sim) * exp(bias)  (expb has zeros cross-window)
                ets = []
                for c in range(2):
                    ps_sim = pst.tile([128, 4, GT], fp32, tag="st")
                    nc.tensor.matmul(
                        ps_sim.rearrange("p a b -> p (a b)"),
                        lhsT=kt[:, c, gq, :],
                        rhs=qbd[par][:, c, :, gq, :],
                        start=True, stop=True,
                    )
                    et = etpool.tile([GT, 4, 128], bf16, tag="et")
                    nc.scalar.activation(out=et[:, :, 0:GT], in_=ps_sim[0:GT],
                                         func=Exp)
                    eng = nc.vector if c == 0 else nc.gpsimd
                    eng.tensor_mul(
                        et[:, :, 0:GT], et[:, :, 0:GT],
                        expb_sb[:, 4 * c:4 * c + 4, :],
                    )
                    ets.append(et)

                # av (8 small matmuls, lhsT 128-col padded for FWL) and the
                # transpose output share ONE packed psum bank tile per group
                pat = pavt.tile([128, 392], fp32, tag="avt")
                pav = pat[:, 0:264].rearrange("p (h c) -> p h c", h=HEADS)
                ptr = pat[:, 264:392].bitcast(bf16).rearrange(
                    "p (a b) -> p a b", a=2)
                for h in range(8):
                    nc.tensor.matmul(
                        pav[:, h, :],
                        lhsT=ets[h // 4][:, h % 4, :],
                        rhs=v1[:, h, :],
                        start=True, stop=True,
                    )

                # normalize: r = 1/s ; ao = av * r (single broadcast multiply)
                r = rpool.tile([GT, HEADS, 1], fp32)
                nc.vector.reciprocal(out=r, in_=pav[0:GT, :, DH:DH + 1])
                ao = aopool.tile([128, 2, 128], bf16, tag="ao")
                aov = ao[0:GT].rearrange("p a (b d) -> p (a b) d", d=DH)
                rb = bass.AP(tensor=r.tensor, offset=r.offset,
                             ap=[r.ap[0], r.ap[1], [0, DH]])
                nc.vector.tensor_mul(aov, pav[0:GT, :, 0:DH], rb)

                # PE-mode transpose ao -> aoT (bf16 PSUM), evac to SBUF
                for kh in range(2):
                    nc.tensor.transpose(ptr[:, kh, :], ao[:, kh, :], ident)
                aot = aotpool.tile([128, 2, 128], bf16)
                nc.vector.tensor_copy(out=aot, in_=ptr)

                # out projection (upper half of the shared pmisc bank tile)
                po = pm[:, 1]
                for kh in range(2):
                    nc.tensor.matmul(
                        po,
                        lhsT=aot[:, kh, :],
                        rhs=wout_sb[:, kh, :],
                        start=(kh == 0),
                        stop=(kh == 1),
                    )
                osb = opool.tile([GT, 256], fp32)
                if gq % 2 == 0:
                    nc.scalar.activation(out=osb, in_=po[0:GT], func=Copy)
                else:
                    nc.vector.tensor_copy(out=osb, in_=po[0:GT])
                nc.sync.dma_start(out=out_d[g], in_=osb)

    nc.compile()
    return nc


def _get_bass(ngroups: int):
    if ngroups not in _CACHE:
        _CACHE[ngroups] = _build_bass(ngroups)
    return _CACHE[ngroups]


def _host_prep(x, w_qkv, w_out, bias_table, rel_pos_indices):
    x = np.asarray(x, dtype=np.float32)
    w_qkv = np.asarray(w_qkv, dtype=np.float32)
    w_out = np.asarray(w_out, dtype=np.float32)
    bias_table = np.asarray(bias_table, dtype=np.float32)
    rel_pos_indices = np.asarray(rel_pos_indices)

    nquads_total = GROUPS_TOTAL // 4
    xg = x.reshape(GROUPS_TOTAL, GT, DIM)
    xT = np.ascontiguousarray(xg.transpose(0, 2, 1)).astype(BF16)
    # [quads, 4(gq), 2(kh), 128(p), GT] -> [quads, 128, 2, 4, GT]
    xT = xT.reshape(nquads_total, 4, 2, 128, GT).transpose(0, 3, 2, 1, 4)
    xT = np.ascontiguousarray(xT)

    wq = w_qkv.copy()
    wq[:, :DIM] *= DH ** -0.5
    wqkv_h = np.ascontiguousarray(wq.reshape(2, 128, 768)).astype(BF16)
    wout_h = np.ascontiguousarray(w_out.reshape(2, 128, 256)).astype(BF16)

    bias = bias_table[rel_pos_indices]        # [60, 60, H]  (i, j, h)
    eb = np.exp(bias.transpose(1, 2, 0))      # [j, h, i]
    expb = np.zeros((GT, HEADS, GT), dtype=np.float32)
    for blk in range(2):
        expb[blk * N_TOK:(blk + 1) * N_TOK, :, blk * N_TOK:(blk + 1) * N_TOK] = eb
    expb_h = expb.astype(BF16)
    return xT, wqkv_h, wout_h, expb_h


def kernel(x, w_qkv, w_out, bias_table, rel_pos_indices):
    global LAST_RESULT
    from concourse.bass_utils import run_bass_kernel_spmd

    xT, wqkv_h, wout_h, expb_h = _host_prep(
        x, w_qkv, w_out, bias_table, rel_pos_indices
    )

    nq = NGROUPS // 4
    nc = _get_bass(NGROUPS)
    in_maps = []
    for c in range(N_CORES):
        in_maps.append({
            "xt": np.ascontiguousarray(xT[c * nq:(c + 1) * nq]),
            "wqkv": wqkv_h,
            "wout": wout_h,
            "expb": expb_h,
            "cachebust": np.zeros((1, KERNEL_REV), dtype=BF16),
        })

    res = run_bass_kernel_spmd(
        nc, in_maps, core_ids=list(range(N_CORES)),
        trace=bool(int(os.environ.get("KERNEL_TRACE", "0"))),
    )
    LAST_RESULT = res

    out = np.concatenate([res.results[c]["out"] for c in range(N_CORES)], axis=0)
    out = out.reshape(16, 16, 16, WH, WW, DIM).astype(np.float32)
    return out
